# revision 69
# baseline (speedup 1.0000x reference)
"""Trainium2 Bass kernel for nn_BasicRecurrentEntityEncoder.

Full-input contract: kernel(**inputs) takes the complete (unsharded) numpy
inputs and returns the full [B, K, D] float32 output. Internally the batch
is sharded over 8 NeuronCores (data parallel, no collectives), the embedding
bag-of-words gather runs through dma_gather against a per-core compacted
bf16 table, and the 64-step entity recurrence runs in a transposed
[D, (b,k)] layout with bf16 matmul operands.

v2 structure (per core: B_local=16, K=32, D=256, S=64, BK=512):
  - gather groups of 128 sentences are software-pipelined INTO the scan:
    group g+1's dma_gather/word-sum/transpose/eW/glk work is emitted in
    chunks between scan steps 8g..8g+7, so only group 0 is a serial prologue.
  - gate constants glk = E^T keys^T are precomputed per gather group with
    the sentence mask AND the block-diagonal batch mask baked in as a -55
    logit offset (sigmoid(-25-) ~ 1e-11 ~ 0), removing all per-step mask ops.
  - per step and batch-group (2 groups of 256 (b,k) columns pipelined):
      psg  = glk-inject + E_t^T h           (64-row block, 3 matmuls, bank B;
                                             glk-inject prefetched at t-1)
      gate = recip1p(exp(-psg))             (Act exp -> custom DVE op)
      gateB = sel64^T gate (dup x2)         (1 matmul, full bank B f32, WAR
                                             after the exp read)
      pshG = kV + eW_bcast + U^T h          (2 prefetched injects + 4 U
                                             matmuls, ONE full-bank group)
      r = relu(pshG); u = r*gateB; upd = u+h; sq = upd^2 (halves)
      ssB  = ones128^T sq (both halves)     (2 matmuls -> [128,256] PSUM,
                                             sum broadcast to ALL partitions
                                             for free: matmul cost = free dim)
      invB = exp(-0.5 ln(ssB+eps))          ([128,256] Act x2, no PE bcast)
      hn   = upd * invB                     (2 DVE ops, halves)
  - mask folding: h_new = normalize(h + (m*gate) .* h_tilda) is exact for
    masked rows because h is always 0 or unit-norm.
"""

import sys

if "/opt/trn_rl_repo" not in sys.path:
    sys.path.insert(0, "/opt/trn_rl_repo")

import numpy as np
import ml_dtypes

from concourse import bacc, mybir
import concourse.bass as bass
import concourse.tile as tile
from concourse.bass_utils import run_bass_kernel_spmd
from concourse.masks import make_identity

# Force every ScalarE activation onto the one table set that covers all the
# functions this kernel uses (relu/exp/ln/copy/identity). The default
# chooser greedily picks the first set per function, inserting a ~1.3us
# table reload per Ln/Exp pair on the critical path.
_ONE_SET = "natural_log_exp_and_others"


import concourse.hw_specs as _hw_specs
_ORIG_TABLES = _hw_specs.get_activation_tables


def _patched_tables(module_arch):
    real = _ORIG_TABLES(module_arch)
    names = list(real.keys())
    assert _ONE_SET in names, names
    out = {}
    for n in names:
        if n == _ONE_SET:
            out[n] = real[n]
            break
        out[n] = set()
    return out


def _install_table_patch():
    import functools
    cached = functools.cache(_patched_tables)
    bacc.get_activation_tables = cached
    _hw_specs.get_activation_tables = cached


_install_table_patch()

# Custom DVE op: out ~= 1/(1 + in0) in ONE VectorE instruction (8 ALU
# stages): u = in0+1; seed y0 = bitcast(~bits(u)); t = u*y0 lands in
# [-4.5, -4] for any positive u; quadratic minimax fixup P(t) ~= 1/t gives
# out = y0*P(t) at ~1e-5 relative error.
import concourse.dve_ops as _dve_ops
from concourse.dve_spec import AluOp as _AluOp, Bin as _Bin, Spec as _Spec
from concourse.dve_spec import C0 as _C0, C1 as _C1, C2 as _C2, One as _One
from concourse.dve_spec import Src0 as _Src0, lower as _dve_lower
from concourse.dve_spec import _has_src1 as _dve_has_src1
from concourse.dve_uop import DveOpSpec as _DveOpSpec


_R1P_C2, _R1P_C1, _R1P_C0 = (lambda c: (c[0], c[1], c[2]))(
    np.polyfit(np.linspace(-4.5, -4.0, 2001),
               1.0 / np.linspace(-4.5, -4.0, 2001), 2))


def _recip1p_ref(in0, in1, c0, c1, c2):
    u = (np.asarray(in0, np.float32) + np.float32(1.0)).astype(np.float32)
    y0 = (~u.view(np.int32)).view(np.float32)
    t = u * y0
    return y0 * (c0 + t * (c1 + c2 * t))


def _make_recip1p():
    u = _Bin(_AluOp.ADD, _Src0, _One)
    y0 = _Bin(_AluOp.BITWISE_NOT, u, u)
    t = u * y0
    spec = _Spec(body=y0 * (_C0 + t * (_C1 + _C2 * t)), reference=_recip1p_ref)
    name = "RECIP1P_APPROX_ANT"
    row = 1 + len(_dve_ops.OPS)
    assert row < 0x20
    shas = {}
    for ver in ("v3", "v4"):
        s = _DveOpSpec(name=name, opcode=row, uops=_dve_lower(spec, ver=ver),
                       rd1_en=_dve_has_src1(spec))
        shas[ver] = s.sha(ver)
    op = _dve_ops.DveOp(name, spec, subdim=False, uops_sha=shas)
    _dve_ops.OPS.append(op)
    _dve_ops._SUB_OPCODE_FOR_NAME[name] = row
    _dve_ops.CUSTOM_DVE_SPECS[name] = spec
    return op


_RECIP1P = _make_recip1p()

F32 = mybir.dt.float32
BF16 = mybir.dt.bfloat16
I16 = mybir.dt.int16
AF = mybir.ActivationFunctionType
OP = mybir.AluOpType

B, S, L, K, D = 128, 64, 32, 32, 256
NC = 8
BL = B // NC              # 16 batch rows per core
BK = BL * K               # 512 = free dim of the state
NG = 8                    # gather groups per core (128 sentences each)
TOKG = 128 * L            # 4096 tokens per group
TABLE_ROWS = 32768        # compacted per-core vocab (unique ids <= 32768)
EPS = 1e-12
HB = BK // 2              # 256 = bk columns per batch-group
SCAN_STEPS = S            # debug hook: emit fewer scan steps

_CACHED = {}
_DBG = {}


def _build_program():
    nc = bacc.Bacc("TRN2", target_bir_lowering=False, debug=False, num_devices=NC)

    table = nc.dram_tensor("table", [TABLE_ROWS, D], BF16, kind="ExternalInput").ap()
    idx16 = nc.dram_tensor("idx16", [128, NG * TOKG // 16], I16, kind="ExternalInput").ap()
    keysT = nc.dram_tensor("keysT", [D, BK], BF16, kind="ExternalInput").ap()
    Umat = nc.dram_tensor("Umat", [D, D], BF16, kind="ExternalInput").ap()
    Vmat = nc.dram_tensor("Vmat", [D, D], BF16, kind="ExternalInput").ap()
    Wmat = nc.dram_tensor("Wmat", [D, D], BF16, kind="ExternalInput").ap()
    glkoff = nc.dram_tensor("glkoff", [128, NG * BK], BF16, kind="ExternalInput").ap()
    selmat = nc.dram_tensor("selmat", [64, 8 * 128], BF16, kind="ExternalInput").ap()
    hout = nc.dram_tensor("hout", [BK, D], F32, kind="ExternalOutput").ap()
    _DBG.clear()
    if SCAN_STEPS == 1:
        for nm, shp in (("dbg_eg", [64, HB]), ("dbg_gm", [64, HB]),
                        ("dbg_r", [128, BK]), ("dbg_u", [128, BK]),
                        ("dbg_inv", [128, HB]), ("dbg_glk", [128, BK])):
            _DBG[nm] = nc.dram_tensor(nm, shp, F32, kind="ExternalOutput").ap()

    with tile.TileContext(nc) as tc:
        _emit(nc, tc, table, idx16, keysT, Umat, Vmat, Wmat, glkoff, selmat, hout)
    nc.compile()
    return nc


def _emit(nc, tc, table, idx16, keysT, Umat, Vmat, Wmat, glkoff, selmat, hout):
    from contextlib import ExitStack

    ctx = ExitStack()
    const = ctx.enter_context(tc.tile_pool(name="const", bufs=1))
    persist = ctx.enter_context(tc.tile_pool(name="persist", bufs=1))
    gpool = ctx.enter_context(tc.tile_pool(name="g", bufs=2))
    work = ctx.enter_context(tc.tile_pool(name="work", bufs=6))
    hpool = ctx.enter_context(tc.tile_pool(name="h", bufs=4))
    # PSUM: 8 banks. psA: pshG per batch-group (2). psB: psm per batch-group
    # x2 bufs (4). psG: gather/output scratch x2 bufs (2).
    psA = ctx.enter_context(tc.tile_pool(name="psA", bufs=1, space="PSUM"))
    psB = ctx.enter_context(tc.tile_pool(name="psB", bufs=2, space="PSUM"))
    psG = ctx.enter_context(tc.tile_pool(name="psG", bufs=2, space="PSUM"))

    # ---- constants into SBUF ----
    sb_idx = const.tile([128, NG * TOKG // 16], I16)
    nc.sync.dma_start(out=sb_idx[:], in_=idx16[:])
    kT = [const.tile([128, BK], BF16, tag=f"kT{j}", name=f"kT{j}") for j in range(2)]
    for j in range(2):
        nc.sync.dma_start(out=kT[j][:], in_=keysT[128 * j:128 * (j + 1), :])
    sbU = [const.tile([128, D], BF16, tag=f"sbU{j}", name=f"sbU{j}") for j in range(2)]
    sbV = [const.tile([128, D], BF16, tag=f"sbV{j}", name=f"sbV{j}") for j in range(2)]
    sbW = [const.tile([128, D], BF16, tag=f"sbW{j}", name=f"sbW{j}") for j in range(2)]
    for j in range(2):
        nc.sync.dma_start(out=sbU[j][:], in_=Umat[128 * j:128 * (j + 1), :])
        nc.sync.dma_start(out=sbV[j][:], in_=Vmat[128 * j:128 * (j + 1), :])
        nc.sync.dma_start(out=sbW[j][:], in_=Wmat[128 * j:128 * (j + 1), :])
    sb_goff = const.tile([128, NG * BK], BF16)
    nc.sync.dma_start(out=sb_goff[:], in_=glkoff[:])

    I128 = const.tile([128, 128], BF16)
    make_identity(nc, I128[:])
    onesF = const.tile([128, 128], BF16)
    nc.vector.memset(onesF[:], 1.0)
    # row selectors for the gate broadcast: sel64[r0//8] sums rows r0..r0+8 of
    # a 64-row operand (the real gate rows inside the 64-aligned psg block;
    # PE operand base partitions must be in {0, 32, 64}). Host-provided since
    # sub-32-aligned partition memsets are rejected by the BIR verifier.
    selsb = const.tile([64, 8 * 128], BF16)
    nc.sync.dma_start(out=selsb[0:64, :], in_=selmat[:])
    sel64 = [selsb[0:64, 128 * i:128 * (i + 1)] for i in range(8)]
    eps128 = const.tile([128, 1], F32)
    nc.vector.memset(eps128[:], EPS)
    # word-sum reducers: Ablk[i][p, m] = 1 iff m == 4*i + p//32.
    Ablk = []
    for i in range(16):
        a = const.tile([128, 64], BF16, tag=f"Ablk{i}", name=f"Ablk{i}")
        nc.vector.memset(a[:], 0.0)
        for q in range(4):
            nc.vector.memset(a[32 * q:32 * (q + 1), 4 * i + q:4 * i + q + 1], 1.0)
        Ablk.append(a)

    # ---- persistent intermediates ----
    ET = [persist.tile([128, NG * 128], BF16, tag=f"ET{j}", name=f"ET{j}") for j in range(2)]
    # eWall[:, m, 128g + s] = (W^T E^T)[de in half m, sentence s of group g]
    eWall = persist.tile([128, 2, NG * 128], BF16, tag="eWall", name="eWall")
    # kvtP[gb] = [V^T keysT half0 | half1] for gb's bk columns: one-matmul inject
    kvtP = [persist.tile([128, BK], BF16, tag=f"kvtP{gb}", name=f"kvtP{gb}")
            for gb in range(2)]
    glk = persist.tile([128, NG * BK], BF16, tag="glk", name="glk")

    # PE p-state warm-up: ~3us of dummy matmuls during the initial DMA wait
    # (the tensor engine ramps low -> mid -> full over ~3us of continuous
    # execution; without this the kVT/word-sum matmuls run 2-4x slow).
    warm = psG.tile([128, 128], F32, tag="g", name="warm")
    for i in range(8):
        nc.tensor.matmul(warm[:], lhsT=onesF[:], rhs=onesF[:],
                         start=(i == 0), stop=(i == 7))

    # kVT = V^T @ keysT, packed per batch-group
    for m in range(2):
        ps = psG.tile([128, BK], F32, tag="g", name="pskv")
        nc.tensor.matmul(ps[:], lhsT=sbV[0][:, 128 * m:128 * (m + 1)], rhs=kT[0][:],
                         start=True, stop=False)
        nc.tensor.matmul(ps[:], lhsT=sbV[1][:, 128 * m:128 * (m + 1)], rhs=kT[1][:],
                         start=False, stop=True)
        for gb in range(2):
            nc.vector.tensor_copy(out=kvtP[gb][:, HB * m:HB * (m + 1)],
                                  in_=ps[:, HB * gb:HB * (gb + 1)])

    # ---- gather-group machinery (emitted in chunks, pipelined into scan) ----
    gstate = {}

    def g_dma(g):
        G = gpool.tile([128, L, D], BF16, tag="G", name=f"G{g}")
        nc.gpsimd.dma_gather(
            out_ap=G[:], in_ap=table[:],
            idxs_ap=sb_idx[:, (TOKG // 16) * g:(TOKG // 16) * (g + 1)],
            num_idxs=TOKG, num_idxs_reg=TOKG, elem_size=D, single_packet=False,
        )
        gstate[g] = {"G": G}

    def g_wordsum(g, c0, c1):
        st = gstate[g]
        if "psE" not in st:
            st["psE"] = psG.tile([128, D], F32, tag="g", name=f"psE{g}")
        psE, G = st["psE"], st["G"]
        for c in range(c0, c1):
            j, i = c // 16, c % 16
            nc.tensor.matmul(psE[64 * j:64 * (j + 1), :], lhsT=Ablk[i][:],
                             rhs=G[:, c, :], start=(i == 0), stop=(i == 15))
        if c1 == L:
            enc = work.tile([128, D], BF16, tag="enc", name=f"enc{g}")
            nc.scalar.copy(out=enc[:], in_=psE[:])
            st["enc"] = enc

    def g_transpose(g):
        enc = gstate[g]["enc"]
        for j in range(2):
            pt = psG.tile([128, 128], BF16, tag="g", name=f"pt{g}_{j}")
            nc.tensor.transpose(pt[:], enc[:, 128 * j:128 * (j + 1)], I128[:])
            nc.vector.tensor_copy(out=ET[j][:, 128 * g:128 * (g + 1)], in_=pt[:])

    def g_ew(g):
        for m in range(2):
            pw = psG.tile([128, 128], F32, tag="g", name=f"pw{g}_{m}")
            nc.tensor.matmul(pw[:], lhsT=sbW[0][:, 128 * m:128 * (m + 1)],
                             rhs=ET[0][:, 128 * g:128 * (g + 1)], start=True, stop=False)
            nc.tensor.matmul(pw[:], lhsT=sbW[1][:, 128 * m:128 * (m + 1)],
                             rhs=ET[1][:, 128 * g:128 * (g + 1)], start=False, stop=True)
            nc.vector.tensor_copy(out=eWall[:, m, 128 * g:128 * (g + 1)], in_=pw[:])

    def g_glk(g):
        pg = psG.tile([128, BK], F32, tag="g", name=f"pglk{g}")
        nc.tensor.matmul(pg[:], lhsT=ET[0][:, 128 * g:128 * (g + 1)], rhs=kT[0][:],
                         start=True, stop=False)
        nc.tensor.matmul(pg[:], lhsT=ET[1][:, 128 * g:128 * (g + 1)], rhs=kT[1][:],
                         start=False, stop=True)
        nc.vector.tensor_tensor(out=glk[:, BK * g:BK * (g + 1)], in0=pg[:],
                                in1=sb_goff[:, BK * g:BK * (g + 1)], op=OP.add)
        del gstate[g]["G"]

    # prologue: group 0 fully, plus group 1's dma
    g_dma(0)
    g_wordsum(0, 0, L)
    g_transpose(0)
    g_ew(0)
    g_glk(0)
    g_dma(1)

    # ---- scan ----
    h = [hpool.tile([128, BK], BF16, tag=f"h{gb}", name=f"h{gb}") for gb in range(2)]
    for gb in range(2):
        nc.vector.memset(h[gb][:], 0.0)

    pshG = [None, None]
    psMn = [None, None]

    def emit_injects(t, gb):
        """Prefetch for step t, batch-group gb (h-independent matmuls):
        bank A: kV + eW broadcast, ONE full-bank accumulation group spanning
        both column halves (closed by the 4 U matmuls inside the step);
        bank B: the glk inject starting the gate-logit group."""
        g, ds = t // 8, t % 8
        off = 16 * ds + 8 * gb
        off64 = (off // 64) * 64
        cg = 128 * g + off
        ps = psA.tile([128, BK], F32, tag=f"psh{gb}", name=f"psh{gb}_{t}")
        pshG[gb] = ps
        nc.tensor.matmul(ps[:, 0:BK], lhsT=I128[:], rhs=kvtP[gb][:],
                         start=True, stop=False)
        ew_bc = (eWall[:, :, cg:cg + 8].unsqueeze(3)
                 .broadcast_to([128, 2, 8, 32]))
        nc.tensor.matmul(ps[:, 0:BK], lhsT=I128[:], rhs=ew_bc,
                         start=False, stop=False)
        psM = psB.tile([128, BK], F32, tag=f"psm{gb}", name=f"psm{gb}_{t}")
        psMn[gb] = psM
        csl = slice(BK * g + HB * gb, BK * g + HB * gb + HB)
        nc.tensor.matmul(psM[0:64, 0:HB],
                         lhsT=I128[off64:off64 + 64, off64:off64 + 64],
                         rhs=glk[off64:off64 + 64, csl], start=True, stop=False)

    emit_injects(0, 0)
    emit_injects(0, 1)

    for t in range(SCAN_STEPS):
        g, ds = t // 8, t % 8
        hn = [None, None]
        # per-gb step state carried across the interleaved emission phases
        stv = [None, None]

        def phase_pe_front(gb):
            """Close the two prefetched accumulation groups with the
            h-dependent matmuls: gate logits first (shortest route to Act),
            then the 4 U matmuls finishing the full-bank pshG group.

            The gate block is computed 64 rows wide (PE operand base
            partitions must be in {0, 64}); only rows r0..r0+8 are the real
            logits, the rest are neighboring steps' garbage that sel64 masks
            out of the broadcast.
            """
            off = 16 * ds + 8 * gb
            off64 = (off // 64) * 64
            cg64 = 128 * g + off64
            hg = h[gb]
            psM = psMn[gb]
            psg = psM[0:64, 0:HB]
            nc.tensor.matmul(psg, lhsT=ET[0][:, cg64:cg64 + 64], rhs=hg[:, 0:HB],
                             start=False, stop=False)
            nc.tensor.matmul(psg, lhsT=ET[1][:, cg64:cg64 + 64], rhs=hg[:, HB:BK],
                             start=False, stop=True)
            ps = pshG[gb]
            for m in range(2):
                msl = slice(HB * m, HB * (m + 1))
                nc.tensor.matmul(ps[:, msl], lhsT=sbU[0][:, 128 * m:128 * (m + 1)],
                                 rhs=hg[:, 0:HB], start=False, stop=False)
                nc.tensor.matmul(ps[:, msl], lhsT=sbU[1][:, 128 * m:128 * (m + 1)],
                                 rhs=hg[:, HB:BK], start=False,
                                 stop=(m == 1))
            stv[gb] = {"psM": psM, "psg": psg}

        def phase_act_gate(gb):
            sv = stv[gb]
            eg = work.tile([64, HB], BF16, tag=f"eg{gb}", name=f"eg{gb}_{t}")
            nc.scalar.activation(eg[:], sv["psg"], AF.Exp, scale=-1.0)
            sv["eg"] = eg

        def phase_dve_gate(gb):
            sv = stv[gb]
            gm = work.tile([64, HB], BF16, tag=f"gm{gb}", name=f"gm{gb}_{t}")
            nc.vector._custom_dve(_RECIP1P, out=gm[:], in0=sv["eg"][:],
                                  s0=float(_R1P_C0), s1=float(_R1P_C1),
                                  imm2=float(_R1P_C2))
            sv["gm"] = gm

        def phase_pe_gateb(gb):
            sv = stv[gb]
            # gate broadcast: dup into both column halves of the bank (full
            # [128, 512] f32, WAR on the exp read of psg), one matmul. sel64
            # picks the 8 real gate rows out of the 64-row block.
            off = 16 * ds + 8 * gb
            r0 = off - (off // 64) * 64
            gB = sv["psM"][:, 0:BK]
            gm_dup = sv["gm"][:].unsqueeze(1).broadcast_to([64, 2, HB])
            nc.tensor.matmul(gB, lhsT=sel64[r0 // 8], rhs=gm_dup,
                             start=True, stop=True)
            sv["gB"] = gB

        def phase_act_relu(gb):
            sv = stv[gb]
            r = work.tile([128, BK], BF16, tag=f"r{gb}", name=f"r{gb}_{t}")
            nc.scalar.activation(r[:], pshG[gb][:], AF.Relu)
            sv["r"] = r

        def phase_dve_mid(gb):
            """u then upd/sq by d-halves so the first ssB matmul can start
            before the second half is squared."""
            sv = stv[gb]
            u = work.tile([128, BK], BF16, tag=f"u{gb}", name=f"u{gb}_{t}")
            nc.vector.tensor_tensor(out=u[:], in0=sv["r"][:], in1=sv["gB"], op=OP.mult)
            upd = work.tile([128, BK], BF16, tag=f"upd{gb}", name=f"upd{gb}_{t}")
            sq = work.tile([128, BK], BF16, tag=f"sq{gb}", name=f"sq{gb}_{t}")
            for half in range(2):
                hs = slice(HB * half, HB * (half + 1))
                nc.vector.tensor_tensor(out=upd[:, hs], in0=u[:, hs],
                                        in1=h[gb][:, hs], op=OP.add)
                nc.vector.tensor_tensor(out=sq[:, hs], in0=upd[:, hs],
                                        in1=upd[:, hs], op=OP.mult)
            sv["upd"], sv["sq"] = upd, sq

        def phase_pe_ss(gb):
            sv = stv[gb]
            ssB = sv["psM"][:, 0:HB]
            nc.tensor.matmul(ssB, lhsT=onesF[:], rhs=sv["sq"][:, 0:HB],
                             start=True, stop=False)
            nc.tensor.matmul(ssB, lhsT=onesF[:], rhs=sv["sq"][:, HB:BK],
                             start=False, stop=True)
            sv["ssB"] = ssB

        def phase_act_norm(gb):
            sv = stv[gb]
            lns = work.tile([128, HB], F32, tag=f"lns{gb}", name=f"lns{gb}_{t}")
            nc.scalar.activation(lns[:], sv["ssB"], AF.Ln, bias=eps128[:])
            invB = work.tile([128, HB], BF16, tag=f"inv{gb}", name=f"inv{gb}_{t}")
            nc.scalar.activation(invB[:], lns[:], AF.Exp, scale=-0.5)
            sv["invB"] = invB

        def phase_dve_norm(gb):
            sv = stv[gb]
            hn[gb] = hpool.tile([128, BK], BF16, tag=f"h{gb}", name=f"hn{gb}_{t}")
            for half in range(2):
                hs = slice(HB * half, HB * (half + 1))
                nc.vector.tensor_tensor(out=hn[gb][:, hs], in0=sv["upd"][:, hs],
                                        in1=sv["invB"][:], op=OP.mult)

        # Interleaved emission: engines execute roughly in program order (the
        # wait/exec queues allow a few instructions of reordering), so
        # per-engine order tracks the temporal order of the two offset
        # chains; each chain's late-input ops are emitted after the other
        # chain's early ops so they don't head-block them.
        phase_pe_front(0)
        phase_act_gate(0)
        phase_dve_gate(0)
        phase_pe_gateb(0)
        phase_act_relu(0)
        phase_dve_mid(0)
        phase_pe_front(1)
        phase_act_gate(1)
        phase_dve_gate(1)
        phase_pe_ss(0)
        phase_pe_gateb(1)
        phase_act_relu(1)
        phase_dve_mid(1)
        if t + 1 < SCAN_STEPS:
            emit_injects(t + 1, 0)
        phase_act_norm(0)
        phase_dve_norm(0)
        phase_pe_ss(1)
        phase_act_norm(1)
        phase_dve_norm(1)
        if t + 1 < SCAN_STEPS:
            emit_injects(t + 1, 1)

        if SCAN_STEPS == 1 and t == 0:
            sv0 = stv[0]
            for nm, src in (("dbg_eg", sv0["eg"][:]), ("dbg_gm", sv0["gm"][:]),
                            ("dbg_r", sv0["r"][:]), ("dbg_u", None),
                            ("dbg_inv", sv0["invB"][:]),
                            ("dbg_glk", glk[:, 0:BK])):
                cp = work.tile(list(_DBG[nm].shape), F32, tag=f"cp{nm}",
                               name=f"cp{nm}")
                if nm == "dbg_u":
                    nc.vector.tensor_copy(out=cp[0:128, :], in_=sv0["gB"])
                else:
                    nc.vector.tensor_copy(out=cp[0:src.shape[0], :], in_=src)
                nc.sync.dma_start(out=_DBG[nm][:], in_=cp[0:_DBG[nm].shape[0], :])

        # pipelined gather work for group g+1, spread across this block
        if ds < 7 or g + 1 < NG:
            gn = g + 1
            if gn < NG:
                if ds == 0:
                    g_wordsum(gn, 0, 8)
                elif ds == 1:
                    g_wordsum(gn, 8, 16)
                elif ds == 2:
                    g_wordsum(gn, 16, 24)
                elif ds == 3:
                    g_wordsum(gn, 24, 32)
                elif ds == 4:
                    g_transpose(gn)
                elif ds == 5:
                    g_ew(gn)
                elif ds == 6:
                    g_glk(gn)
                elif ds == 7 and gn + 1 < NG:
                    g_dma(gn + 1)

        h = hn

    # ---- output: transpose h^T [256, 512] -> [512, 256] fp32 ----
    for q in range(4):
        gb, half = q // 2, q % 2
        ho = work.tile([128, D], F32, tag="ho")
        for j in range(2):
            pt = psG.tile([128, 128], BF16, tag="g", name="ptout")
            nc.tensor.transpose(pt[:], h[gb][:, HB * j + 128 * half:
                                             HB * j + 128 * half + 128], I128[:])
            nc.vector.tensor_copy(out=ho[:, 128 * j:128 * (j + 1)], in_=pt[:])
        nc.sync.dma_start(out=hout[128 * q:128 * (q + 1), :], in_=ho[:])

    ctx.close()


def _prep_core(pr, mask, keys_c, emb):
    """Host-side marshaling for one core's shard."""
    uniq, inv = np.unique(pr, return_inverse=True)
    assert len(uniq) <= TABLE_ROWS
    table = np.zeros((TABLE_ROWS, D), dtype=ml_dtypes.bfloat16)
    table[: len(uniq)] = emb[uniq].astype(ml_dtypes.bfloat16)
    ranks = inv.reshape(BL, S, L).astype(np.int16)

    # token order per group g: i = (ds*16 + b)*32 + w
    idx_groups = []
    for g in range(NG):
        blk = ranks[:, 8 * g:8 * (g + 1), :]          # [b, ds, w]
        lst = blk.transpose(1, 0, 2).reshape(-1)      # [(ds, b, w)] length 4096
        idx_groups.append(np.tile(lst.reshape(TOKG // 16, 16).T, (8, 1)))
    idx16 = np.concatenate(idx_groups, axis=1).astype(np.int16)  # [128, NG*256]

    keysT = np.ascontiguousarray(
        keys_c.reshape(BK, D).T).astype(ml_dtypes.bfloat16)      # [256, 512]

    # glkoff[16*ds + b, 32*b' + k] for group g: 0 where (b'==b and
    # mask[b, 8g+ds]), else -55 (drives sigmoid to ~0, folding both the
    # block-diagonal select and the sentence mask into the gate logits).
    ds_idx = np.arange(128) // 16
    b_idx = np.arange(128) % 16
    bp = np.arange(BK) // K
    diag = (b_idx[:, None] == bp[None, :])            # [128, 512]
    glkoff = np.empty((128, NG * BK), dtype=np.float32)
    for g in range(NG):
        m_g = mask[b_idx, 8 * g + ds_idx]             # [128]
        ok = diag & m_g[:, None]
        glkoff[:, BK * g:BK * (g + 1)] = np.where(ok, 0.0, -55.0)
    return table, idx16, keysT, glkoff.astype(ml_dtypes.bfloat16)


def kernel(prgrph, prgrph_mask, keys, embedding_matrix, U, V, W):
    prgrph = np.asarray(prgrph)
    prgrph_mask = np.asarray(prgrph_mask)
    keys = np.asarray(keys, dtype=np.float32)
    emb = np.asarray(embedding_matrix, dtype=np.float32)
    U = np.asarray(U, dtype=np.float32)
    V = np.asarray(V, dtype=np.float32)
    W = np.asarray(W, dtype=np.float32)

    if "nc" not in _CACHED:
        _CACHED["nc"] = _build_program()
    nc = _CACHED["nc"]

    Ub, Vb, Wb = (x.astype(ml_dtypes.bfloat16) for x in (U, V, W))
    selmat = np.zeros((64, 8 * 128), dtype=ml_dtypes.bfloat16)
    for i in range(8):
        selmat[8 * i:8 * i + 8, 128 * i:128 * (i + 1)] = 1.0

    in_maps = []
    for c in range(NC):
        sl = slice(BL * c, BL * (c + 1))
        table, idx16, keysT, glkoff = _prep_core(
            prgrph[sl], prgrph_mask[sl, :, 0], keys[sl], emb)
        in_maps.append({
            "table": table, "idx16": idx16, "keysT": keysT,
            "Umat": Ub, "Vmat": Vb, "Wmat": Wb,
            "glkoff": glkoff, "selmat": selmat,
        })

    res = run_bass_kernel_spmd(nc, in_maps, core_ids=list(range(NC)))
    _CACHED["res"] = res
    out = np.concatenate(
        [res.results[c]["hout"].reshape(BL, K, D) for c in range(NC)], axis=0)
    return out.astype(np.float32)
